# revision 30
# baseline (speedup 1.0000x reference)
"""Trainium2 Bass kernel: batched single-head attention + residual + layernorm.

Reference (per batch element b of 8, one NeuronCore each — data-parallel):
    q = X@Wq+bq; k = X@Wk+bk; v = X@Wv+bv          [S=2048, K=64]
    attn = softmax(q @ k.T / 8, axis=-1)            [S, S]
    y = X + (attn @ v) @ Wo + bo                    [S, D=1024]
    out = layernorm(y) * gamma + beta

v4 design (fp8 DoubleRow everywhere it pays + matmul-based LN stats):
  Host algebra: bk drops out of softmax (constant along keys); bo'=bo+bv@Wo;
  r solves r(I + Wv@Wo) = bo' so ONE shifted input X+r serves both the
  projections (q bias corrected by -r@Wq; k/v shifts cancel exactly) and the
  residual XB = bf16(X+r). Host uploads XT8 = fp8(X+r) pre-transposed in
  DoubleRow pair layout [128,c,i,s] (d = 2(128c+p)+i), XB, DR-packed fp8
  weights, and rowsum(XB^2). OUT returns bf16, upcast on host.

  Device per core:
  1. q/k projection: 4 fp8 DR matmuls -> psqk. Block 0 is redistributed into
     q8/k8 [32,2,S] DR layouts by 4 direct DVE ops (jumpstart, hidden in the
     startup bubble); blocks 1-3 go via one DVE cast to qkT8 fp8 + tiny
     SBUF->SBUF DMAs. kappa = 32i+p.
  2. v projection: DR matmuls with lhsT=XT8 s-slice emit v in natural [s,K]
     layout; DVE-copied to bf16 v8 with a ones column (softmax sums).
  3. scores: one DR matmul per 128-key chunk (contraction 2x32); exp on ACT
     in [128,1024] two-chunk units, bias -ln2 (fp8 range guard), bf16 expT.
     Emission order keeps block-0 units ahead of block-1 in the ACT queue.
  4. uav: bf16 matmuls accumulate the attention numerator AND softmax sums
     (ones column) in one PSUM group; uavT8 = DVE fp8 cast (division is
     deferred into the residual pass); sums row ACT-copied to SBUF,
     PE-transposed to per-query columns, reciprocal + recip^2 on DVE.
  5. y: Wo matmul as fp8 DR from avT8; residual + softmax division fused in
     one DVE scalar_tensor_tensor per psy half, accum = rowsum(y) -> mean.
  6. Variance WITHOUT a quadratic pass over y: rowsum(y^2) = rowsum(XB^2)
     [host] + 2 recip (uav . h) + recip^2 (uav . G uav), where h = Wo@XB
     comes from tiny DR matmuls against XT8, G = Wo@Wo.T = L L.T (host
     Cholesky), z = L.T uav from one tiny DR matmul. The two dots are 192ns
     DVE scalar_tensor_tensor ops over [128,64] PSUM with recip/recip^2 in
     the scalar slot. Newton rsqrt + variance combine on GPSIMD (pool);
     normalize on DVE at 4x (bf16 SBUF); last block runs stats on DVE to
     shorten the drain tail. Output bf16.
"""

import numpy as np
import ml_dtypes

B = 8
S = 2048
D = 1024
K = 64
EPS = 1e-5
NB = 4          # 512-query blocks
NT = 16         # 128-row tiles
NC = 4          # 256-deep contraction chunks (DoubleRow pairs of 128)

F8 = ml_dtypes.float8_e4m3
BF = ml_dtypes.bfloat16

_COMPILED = {}


def _build_bass():
    import concourse.bacc as bacc
    import concourse.tile as tile
    from concourse import mybir

    f32 = mybir.dt.float32
    bf16 = mybir.dt.bfloat16
    fp8 = mybir.dt.float8e4
    AF = mybir.ActivationFunctionType
    alu = mybir.AluOpType
    DRm = mybir.MatmulPerfMode.DoubleRow

    nc = bacc.Bacc("TRN2", target_bir_lowering=False, debug=False)

    xt8_d = nc.dram_tensor("XT8", [128, NC, 2, S], fp8, kind="ExternalInput")
    xb_d = nc.dram_tensor("XB", [S, D], bf16, kind="ExternalInput")
    xb2_d = nc.dram_tensor("XB2", [128, NT], f32, kind="ExternalInput")
    xbs_d = nc.dram_tensor("XBS", [128, NT], f32, kind="ExternalInput")
    wqk_d = nc.dram_tensor("WQK8", [128, NC, 2, 128], fp8, kind="ExternalInput")
    wv_d = nc.dram_tensor("WV8", [128, NC, 2, K], fp8, kind="ExternalInput")
    wo_d = nc.dram_tensor("WOB8", [32, 2, D], fp8, kind="ExternalInput")
    wot_d = nc.dram_tensor("WOT8", [128, NC, 2, K], fp8, kind="ExternalInput")
    il_d = nc.dram_tensor("IL8", [32, 2, 2 * K + 1], fp8, kind="ExternalInput")
    bqk_d = nc.dram_tensor("BQK", [128, 1], f32, kind="ExternalInput")
    out_d = nc.dram_tensor("OUT", [S, D], bf16, kind="ExternalOutput")

    LN2 = 0.6931471805599453

    with tile.TileContext(nc) as tc:
        with (
            tc.tile_pool(name="consts", bufs=1) as consts,
            tc.tile_pool(name="bigx", bufs=1) as bigx,
            tc.tile_pool(name="proj", bufs=1) as proj,
            tc.tile_pool(name="expp", bufs=3) as expp,
            tc.tile_pool(name="ysb", bufs=6) as ysb,
            tc.tile_pool(name="y0p", bufs=3) as y0p,
            tc.tile_pool(name="scrp", bufs=1) as scrp,
            tc.tile_pool(name="outp", bufs=4) as outp,
            tc.tile_pool(name="work", bufs=4) as work,
            tc.tile_pool(name="srp", bufs=2) as srp,
            # PSUM rings: "pss" scores units (4 banks), "psu" uav
            # accumulators+psums (2 banks), "h" psqk/psv/psy-halves/psq3
            # (2 banks).
            tc.tile_pool(name="pss", bufs=2, space="PSUM") as psS,
            tc.tile_pool(name="psu", bufs=2, space="PSUM") as psU,
            tc.tile_pool(name="psh", bufs=2, space="PSUM") as psH,
        ):
            # ---- weights first (projections block on them), then XT8.
            wqk8 = consts.tile([128, NC, 2, 128], fp8)
            nc.sync.dma_start(out=wqk8, in_=wqk_d[:])
            bqk_col = consts.tile([128, 1], f32)
            nc.sync.dma_start(out=bqk_col, in_=bqk_d[:])
            xt8 = bigx.tile([128, NC, 2, S], fp8)
            for h in range(2):
                for c in range(NC):
                    nc.sync.dma_start(
                        out=xt8[:, c, :, h * 1024:(h + 1) * 1024],
                        in_=xt8_d[:, c, :, h * 1024:(h + 1) * 1024],
                    )
            wv8 = consts.tile([128, NC, 2, K], fp8)
            wob8 = consts.tile([32, 2, D], fp8)
            wot8 = consts.tile([128, NC, 2, K], fp8)
            il8 = consts.tile([32, 2, 2 * K + 1], fp8)
            xb2s = consts.tile([128, NT], f32)
            xbs_s = consts.tile([128, NT], f32)
            xb = bigx.tile([128, NT, D], bf16)
            xb_view = xb_d[:].rearrange("(t p) d -> p t d", p=128)

            ones16 = consts.tile([128, NT], bf16)
            nc.vector.memset(ones16, 1.0)
            nln2 = consts.tile([128, 1], f32)
            nc.vector.memset(nln2, -LN2)

            qkT8 = proj.tile([128, S], fp8)
            q8 = proj.tile([32, 2, S], fp8)
            k8 = proj.tile([32, 2, S], fp8)
            vh8 = proj.tile([128, NT, 2 * K], bf16)  # v | h=Wo@XB per tile
            avT8 = proj.tile([32, 2, S], fp8)
            uavT8 = proj.tile([K, S], fp8)
            rec_sb = work.tile([128, NT], f32)
            rec2_sb = work.tile([128, NT], f32)
            musum = work.tile([128, NT], f32)      # rowsum(y) via wdot trick
            csum = work.tile([128, NT], f32)       # recip * (uav . h)
            qsum = work.tile([128, NT], f32)       # recip^2 * |L.T uav|^2
            scr = scrp.tile([128, 2 * K], bf16)
            zs = scrp.tile([128, K], bf16)

            live = {}

            def emit_scores(tgt, units):
                """Scores + exp for query block tgt; units of 2 key-chunks."""
                if tgt not in live:
                    live[tgt] = expp.tile(
                        [128, NT, 512], bf16, tag="expT", name=f"expT{tgt}"
                    )
                et = live[tgt]
                sq = slice(tgt * 512, (tgt + 1) * 512)
                for u in units:
                    pss = psS.tile([128, 1024], f32, tag="pss", name=f"pss{tgt}_{u}")
                    for j in range(2):
                        sk = 2 * u + j
                        nc.tensor.matmul(
                            pss[:, j * 512:(j + 1) * 512],
                            k8[:, :, sk * 128:(sk + 1) * 128],
                            q8[:, :, sq],
                            start=True, stop=True, perf_mode=DRm,
                        )
                    nc.scalar.activation(
                        out=et[:, 2 * u:2 * u + 2, :].rearrange("p u q -> p (u q)"),
                        in_=pss,
                        func=AF.Exp, scale=0.125, bias=nln2,
                    )

            # ---- phase 1a: q/k projections + redistribution (compact
            # DVE TSP chain; v-projection deferred so the psH ring only
            # rotates psqk tiles here)
            for b in range(NB):
                sq = slice(b * 512, (b + 1) * 512)
                psqk = psH.tile([128, 512], f32, tag="h", name=f"psqk{b}")
                for c in range(NC):
                    nc.tensor.matmul(
                        psqk, wqk8[:, c], xt8[:, c, :, sq],
                        start=(c == 0), stop=(c == NC - 1), perf_mode=DRm,
                    )
                nc.vector.tensor_scalar(
                    out=qkT8[:, sq], in0=psqk, scalar1=bqk_col, scalar2=None,
                    op0=alu.add,
                )
                # redistribution littles (kappa = 32i+p); k8 gates score
                # emission so it lands per-block; q8 for blocks 2-3 is
                # phase-2-only and goes as one pair.
                nc.sync.dma_start(out=k8[:, 0, sq], in_=qkT8[64:96, sq])
                nc.sync.dma_start(out=k8[:, 1, sq], in_=qkT8[96:128, sq])
                if b <= 1:
                    nc.sync.dma_start(out=q8[:, 0, sq], in_=qkT8[0:32, sq])
                    nc.sync.dma_start(out=q8[:, 1, sq], in_=qkT8[32:64, sq])
                if b == 3:
                    sp = slice(1024, 2048)
                    nc.sync.dma_start(out=q8[:, 0, sp], in_=qkT8[0:32, sp])
                    nc.sync.dma_start(out=q8[:, 1, sp], in_=qkT8[32:64, sp])

                # keep block-0 exp units strictly ahead of block-1 in the
                # ACT queue; each unit is emitted after its k8 cols land.
                if b == 0:
                    emit_scores(0, [0, 1])
                elif b == 1:
                    emit_scores(0, [2, 3])
                elif b == 2:
                    emit_scores(0, [4, 5])
                else:
                    emit_scores(0, [6, 7])
                    emit_scores(1, range(0, 8))

            # deferred small weight loads (behind the phase-1a littles in
            # the SP queue; all are first needed well after them)
            nc.sync.dma_start(out=wv8, in_=wv_d[:])
            nc.sync.dma_start(out=wob8, in_=wo_d[:])
            nc.sync.dma_start(out=wot8, in_=wot_d[:])
            nc.sync.dma_start(out=il8, in_=il_d[:])
            nc.sync.dma_start(out=xb2s, in_=xb2_d[:])
            nc.sync.dma_start(out=xbs_s, in_=xbs_d[:])

            # ---- phase 1b: v projection (needed first at uav of block 0)
            for b in range(NB):
                psv = psH.tile([128, 4, 2 * K], f32, tag="h", name=f"psv{b}")
                for t in range(4):
                    sk = slice((4 * b + t) * 128, (4 * b + t + 1) * 128)
                    for c in range(NC):
                        nc.tensor.matmul(
                            psv[:, t, 0:K], xt8[:, c, :, sk], wv8[:, c],
                            start=(c == 0), stop=(c == NC - 1), perf_mode=DRm,
                        )
                    for c in range(NC):
                        nc.tensor.matmul(
                            psv[:, t, K:2 * K], xt8[:, c, :, sk], wot8[:, c],
                            start=(c == 0), stop=(c == NC - 1), perf_mode=DRm,
                        )
                nc.vector.tensor_copy(out=vh8[:, 4 * b:4 * b + 4, :], in_=psv)

            # XB loads ride the gpsimd SWDGE queue (pool is idle early),
            # gated on the LAST redistribution little (q8 block-3 pair) via
            # tiny fake writes so the transfers never block phase-1 DMAs.
            for g in range(8):
                nc.gpsimd.tensor_copy(
                    out=xb[0:1, 2 * g, 0:1], in_=q8[0:1, 0, S - 1:S]
                )
                nc.gpsimd.dma_start(
                    out=xb[:, 2 * g:2 * g + 2, :], in_=xb_view[:, 2 * g:2 * g + 2, :]
                )

            # ---- phase 2: uav + y + layernorm
            out_view = out_d[:].rearrange("(t p) d -> p t d", p=128)
            for b in range(NB):
                sq = slice(b * 512, (b + 1) * 512)
                expT = live.pop(b)
                psu = psU.tile([K, 512], f32, tag="psu", name=f"psuav{b}")
                for sk in range(NT):
                    nc.tensor.matmul(
                        psu, vh8[:, sk, 0:K], expT[:, sk, :],
                        start=(sk == 0), stop=(sk == NT - 1),
                    )
                if b < NB - 1:
                    nc.vector.tensor_copy(out=uavT8[:, sq], in_=psu)
                    nc.scalar.dma_start(out=avT8[:, 0, sq], in_=uavT8[0:32, sq])
                    nc.scalar.dma_start(out=avT8[:, 1, sq], in_=uavT8[32:64, sq])
                else:
                    nc.vector.tensor_copy(out=avT8[:, 0, sq], in_=psu[0:32, :])
                    nc.vector.tensor_copy(out=avT8[:, 1, sq], in_=psu[32:64, :])
                # softmax sums per query column via tiny PE accumulations
                psums = psU.tile([128, 4], f32, tag="psu", name=f"psums{b}")
                for j in range(4):
                    for sk in range(NT):
                        nc.tensor.matmul(
                            psums[:, j:j + 1],
                            expT[:, sk, j * 128:(j + 1) * 128],
                            ones16[:, 0:1],
                            start=(sk == 0), stop=(sk == NT - 1),
                        )
                bs = slice(4 * b, 4 * b + 4)
                nc.vector.reciprocal(out=rec_sb[:, bs], in_=psums)
                nc.vector.tensor_tensor(
                    out=rec2_sb[:, bs], in0=rec_sb[:, bs], in1=rec_sb[:, bs],
                    op=alu.mult,
                )

                if b + 2 < NB:
                    emit_scores(b + 2, range(8))

                last = b == NB - 1

                def emit_ti(t, act_assist):
                    yt = ysb.tile([128, D], bf16, tag="y", name=f"y{t}")
                    if act_assist:
                        y0 = y0p.tile([128, D], bf16, tag="y0", name=f"y0_{t}")
                    for j in range(2):
                        psy = psH.tile([128, 512], f32, tag="h", name=f"psy{t}_{j}")
                        nc.tensor.matmul(
                            psy,
                            avT8[:, :, t * 128:(t + 1) * 128],
                            wob8[:, :, j * 512:(j + 1) * 512],
                            start=True, stop=True, perf_mode=DRm,
                        )
                        if act_assist:
                            # ACT is past its exp stream here: scale on ACT,
                            # residual add on DVE at 2x (all-SBUF bf16)
                            nc.scalar.mul(
                                out=y0[:, j * 512:(j + 1) * 512], in_=psy,
                                mul=rec_sb[:, t:t + 1],
                            )
                        else:
                            nc.vector.scalar_tensor_tensor(
                                out=yt[:, j * 512:(j + 1) * 512],
                                in0=psy, scalar=rec_sb[:, t:t + 1],
                                in1=xb[:, t, j * 512:(j + 1) * 512],
                                op0=alu.mult, op1=alu.add,
                            )
                    if act_assist:
                        nc.vector.tensor_tensor(
                            out=yt, in0=y0, in1=xb[:, t, :], op=alu.add,
                        )
                    # LN pieces: uav-nat | z = L.T uav | wdot (h = Wo@XB
                    # precomputed in phase 1b; one PSUM input per DVE op)
                    psq3 = psH.tile([128, 2 * K + 1], f32, tag="h",
                                    name=f"psq3{t}")
                    tsl = slice(t * 128, (t + 1) * 128)
                    nc.tensor.matmul(
                        psq3, avT8[:, :, tsl], il8,
                        start=True, stop=True, perf_mode=DRm,
                    )
                    nc.vector.scalar_tensor_tensor(
                        out=scr[:, 0:K], in0=psq3[:, 0:K],
                        scalar=rec_sb[:, t:t + 1], in1=vh8[:, t, K:2 * K],
                        op0=alu.mult, op1=alu.mult,
                        accum_out=csum[:, t:t + 1],
                    )
                    nc.vector.tensor_scalar(
                        out=zs, in0=psq3[:, K:2 * K],
                        scalar1=rec_sb[:, t:t + 1], scalar2=None, op0=alu.mult,
                    )
                    nc.vector.scalar_tensor_tensor(
                        out=scr[:, K:2 * K], in0=zs, scalar=1.0, in1=zs,
                        op0=alu.mult, op1=alu.mult,
                        accum_out=qsum[:, t:t + 1],
                    )
                    # rowsum(y) = xbs + rec*wdot  (no accumulator needed)
                    nc.vector.scalar_tensor_tensor(
                        out=musum[:, t:t + 1], in0=psq3[:, 2 * K:2 * K + 1],
                        scalar=rec_sb[:, t:t + 1], in1=xbs_s[:, t:t + 1],
                        op0=alu.mult, op1=alu.add,
                    )
                    live[f"y{t}"] = yt

                def emit_group(t0, n, eng, neng):
                    ts = slice(t0, t0 + n)
                    mu = work.tile([128, n], f32, tag="mu")
                    eng.tensor_scalar(
                        out=mu, in0=musum[:, ts], scalar1=1.0 / D, scalar2=None,
                        op0=alu.mult,
                    )
                    # ve = (xb2 + 2*csum + qsum)/D + EPS - mu^2
                    ve = work.tile([128, n], f32, tag="ve")
                    eng.tensor_tensor(out=ve, in0=csum[:, ts], in1=csum[:, ts],
                                      op=alu.add)
                    eng.tensor_tensor(out=ve, in0=ve, in1=qsum[:, ts], op=alu.add)
                    eng.tensor_tensor(out=ve, in0=ve, in1=xb2s[:, ts], op=alu.add)
                    eng.tensor_scalar(
                        out=ve, in0=ve, scalar1=1.0 / D, scalar2=EPS,
                        op0=alu.mult, op1=alu.add,
                    )
                    mu2 = work.tile([128, n], f32, tag="mu2")
                    eng.tensor_tensor(out=mu2, in0=mu, in1=mu, op=alu.mult)
                    eng.tensor_tensor(out=ve, in0=ve, in1=mu2, op=alu.subtract)
                    # rstd = rsqrt(ve) via multiply-only Newton (w ~ [0.7, 1.4])
                    rstd = work.tile([128, n], f32, tag="rstd")
                    eng.tensor_scalar(
                        out=rstd, in0=ve, scalar1=-0.5, scalar2=1.5,
                        op0=alu.mult, op1=alu.add,
                    )
                    for _ in range(2):
                        na = work.tile([128, n], f32, tag="na")
                        eng.tensor_tensor(out=na, in0=rstd, in1=rstd, op=alu.mult)
                        eng.tensor_tensor(out=na, in0=na, in1=ve, op=alu.mult)
                        eng.tensor_scalar(
                            out=na, in0=na, scalar1=-0.5, scalar2=1.5,
                            op0=alu.mult, op1=alu.add,
                        )
                        rstd2 = work.tile([128, n], f32, tag="rstd")
                        eng.tensor_tensor(out=rstd2, in0=rstd, in1=na, op=alu.mult)
                        rstd = rstd2
                    for t in range(t0, t0 + n):
                        yt = live.pop(f"y{t}")
                        ot = outp.tile([128, D], bf16, tag="o")
                        neng.tensor_scalar(
                            out=ot, in0=yt,
                            scalar1=mu[:, t - t0:t - t0 + 1],
                            scalar2=rstd[:, t - t0:t - t0 + 1],
                            op0=alu.subtract, op1=alu.mult,
                        )
                        nc.sync.dma_start(out=out_view[:, t, :], in_=ot)

                if b <= 1:
                    for t in range(4 * b, 4 * b + 4):
                        emit_ti(t, False)
                    emit_group(4 * b, 4, nc.gpsimd, nc.gpsimd)
                elif b == 2:
                    for t in range(8, 12):
                        emit_ti(t, False)
                    emit_group(8, 4, nc.gpsimd, nc.vector)
                else:
                    # pair-interleaved drain on DVE+ACT
                    emit_ti(12, True)
                    emit_ti(13, True)
                    emit_group(12, 2, nc.vector, nc.vector)
                    emit_ti(14, True)
                    emit_ti(15, True)
                    emit_group(14, 2, nc.vector, nc.vector)

    nc.compile()
    return nc


def _get_compiled():
    if "nc" not in _COMPILED:
        _COMPILED["nc"] = _build_bass()
    return _COMPILED["nc"]


def _prep_weights(Wq, bq, Wk, bk, Wv, bv, Wo, bo):
    """Host-side folding + DR packing. d = 2*(128c+p)+i for contractions."""
    bo_f = bo + bv @ Wo
    # r(I + Wv@Wo) = bo'  ->  the single input shift serving proj + residual
    r = np.linalg.solve((np.eye(D, dtype=np.float64) + Wv.astype(np.float64)
                         @ Wo.astype(np.float64)).T, bo_f.astype(np.float64))
    r = r.astype(np.float32)
    bq_f = bq - r @ Wq

    wqk = np.concatenate([Wq, Wk], axis=1)  # [D, 128]
    p = np.arange(128)
    wqk8 = np.zeros((128, NC, 2, 128), dtype=F8)
    wv8 = np.zeros((128, NC, 2, K), dtype=F8)
    wot8 = np.zeros((128, NC, 2, K), dtype=F8)
    WoT = np.ascontiguousarray(Wo.T)  # [D, K]
    for c in range(NC):
        for i in range(2):
            ds = 2 * (128 * c + p) + i
            wqk8[:, c, i, :] = wqk[ds, :].astype(F8)
            wv8[:, c, i, :] = Wv[ds, :].astype(F8)
            wot8[:, c, i, :] = WoT[ds, :].astype(F8)
    # kappa = 32i + p: Wo rows [0:32] -> plane 0, [32:64] -> plane 1
    wob8 = np.zeros((32, 2, D), dtype=F8)
    wob8[:, 0, :] = Wo[0:32, :].astype(F8)
    wob8[:, 1, :] = Wo[32:64, :].astype(F8)
    # il8 = [I | L | worow] for the uav-nat, z = L.T uav, wdot matmuls
    G = (Wo @ Wo.T).astype(np.float64)
    L = np.linalg.cholesky(G + 1e-9 * np.eye(K))
    il = np.zeros((K, 2 * K + 1), dtype=np.float32)
    il[:, 0:K] = np.eye(K, dtype=np.float32)
    il[:, K:2 * K] = L.astype(np.float32)
    il[:, 2 * K] = Wo.sum(axis=1)
    il8 = np.zeros((32, 2, 2 * K + 1), dtype=F8)
    il8[:, 0, :] = il[0:32, :].astype(F8)
    il8[:, 1, :] = il[32:64, :].astype(F8)
    bqk = np.zeros((128, 1), dtype=np.float32)
    bqk[0:K, 0] = bq_f
    return wqk8, wv8, wob8, wot8, il8, bqk, r


def kernel(X, Wq, bq, Wk, bk, Wv, bv, Wo, bo, gamma, beta):
    from concourse.bass_utils import run_bass_kernel_spmd

    X = np.ascontiguousarray(np.asarray(X, dtype=np.float32))
    Wq, bq = np.asarray(Wq, np.float32), np.asarray(bq, np.float32)
    Wk, bk = np.asarray(Wk, np.float32), np.asarray(bk, np.float32)
    Wv, bv = np.asarray(Wv, np.float32), np.asarray(bv, np.float32)
    Wo, bo = np.asarray(Wo, np.float32), np.asarray(bo, np.float32)
    gamma_np = np.asarray(gamma, dtype=np.float32)
    beta_np = np.asarray(beta, dtype=np.float32)

    wqk8, wv8, wob8, wot8, il8, bqk, r = _prep_weights(
        Wq, bq, Wk, bk, Wv, bv, Wo, bo
    )

    nc = _get_compiled()
    in_maps = []
    for bi in range(B):
        Xr = X[bi] + r
        X8 = Xr.astype(F8)
        # XT8[p, c, i, s] = X8[s, 256c + 2p + i]
        xt8 = np.ascontiguousarray(
            X8.reshape(S, NC, 128, 2).transpose(2, 1, 3, 0)
        )
        xbi = Xr.astype(BF)
        xbf = xbi.astype(np.float32)
        xb2 = (xbf ** 2).sum(axis=1)  # [S]
        xb2t = np.ascontiguousarray(xb2.reshape(NT, 128).T)  # [128, NT]
        xbs = xbf.sum(axis=1)
        xbst = np.ascontiguousarray(xbs.reshape(NT, 128).T)
        in_maps.append({
            "XT8": xt8, "XB": xbi, "XB2": xb2t, "XBS": xbst,
            "WQK8": wqk8, "WV8": wv8, "WOB8": wob8, "WOT8": wot8,
            "IL8": il8, "BQK": bqk,
        })
    res = run_bass_kernel_spmd(nc, in_maps, core_ids=list(range(B)))
    out = np.stack(
        [np.asarray(res.results[i]["OUT"], dtype=np.float32) for i in range(B)],
        axis=0,
    )
    if not (np.all(gamma_np == 1.0) and np.all(beta_np == 0.0)):
        out = out * gamma_np + beta_np
    return out.astype(np.float32)


# revision 37
# speedup vs baseline: 1.0285x; 1.0285x over previous
"""Trainium2 Bass kernel: batched single-head attention + residual + layernorm.

Reference (per batch element b of 8, one NeuronCore each — data-parallel):
    q = X@Wq+bq; k = X@Wk+bk; v = X@Wv+bv          [S=2048, K=64]
    attn = softmax(q @ k.T / 8, axis=-1)            [S, S]
    y = X + (attn @ v) @ Wo + bo                    [S, D=1024]
    out = layernorm(y) * gamma + beta

v4 design (fp8 DoubleRow everywhere it pays + matmul-based LN stats):
  Host algebra: bk drops out of softmax (constant along keys); bo'=bo+bv@Wo;
  r solves r(I + Wv@Wo) = bo' so ONE shifted input X+r serves both the
  projections (q bias corrected by -r@Wq; k/v shifts cancel exactly) and the
  residual XB = bf16(X+r). Host uploads XT8 = fp8(X+r) pre-transposed in
  DoubleRow pair layout [128,c,i,s] (d = 2(128c+p)+i), XB, DR-packed fp8
  weights, and rowsum(XB^2). OUT returns bf16, upcast on host.

  Device per core:
  1. q/k projection: 4 fp8 DR matmuls -> psqk. Block 0 is redistributed into
     q8/k8 [32,2,S] DR layouts by 4 direct DVE ops (jumpstart, hidden in the
     startup bubble); blocks 1-3 go via one DVE cast to qkT8 fp8 + tiny
     SBUF->SBUF DMAs. kappa = 32i+p.
  2. v projection: DR matmuls with lhsT=XT8 s-slice emit v in natural [s,K]
     layout; DVE-copied to bf16 v8 with a ones column (softmax sums).
  3. scores: one DR matmul per 128-key chunk (contraction 2x32); exp on ACT
     in [128,1024] two-chunk units, bias -ln2 (fp8 range guard), bf16 expT.
     Emission order keeps block-0 units ahead of block-1 in the ACT queue.
  4. uav: bf16 matmuls accumulate the attention numerator AND softmax sums
     (ones column) in one PSUM group; uavT8 = DVE fp8 cast (division is
     deferred into the residual pass); sums row ACT-copied to SBUF,
     PE-transposed to per-query columns, reciprocal + recip^2 on DVE.
  5. y: Wo matmul as fp8 DR from avT8; residual + softmax division fused in
     one DVE scalar_tensor_tensor per psy half, accum = rowsum(y) -> mean.
  6. Variance WITHOUT a quadratic pass over y: rowsum(y^2) = rowsum(XB^2)
     [host] + 2 recip (uav . h) + recip^2 (uav . G uav), where h = Wo@XB
     comes from tiny DR matmuls against XT8, G = Wo@Wo.T = L L.T (host
     Cholesky), z = L.T uav from one tiny DR matmul. The two dots are 192ns
     DVE scalar_tensor_tensor ops over [128,64] PSUM with recip/recip^2 in
     the scalar slot. Newton rsqrt + variance combine on GPSIMD (pool);
     normalize on DVE at 4x (bf16 SBUF); last block runs stats on DVE to
     shorten the drain tail. Output bf16.
"""

import numpy as np
import ml_dtypes

B = 8
S = 2048
D = 1024
K = 64
EPS = 1e-5
NB = 4          # 512-query blocks
NT = 16         # 128-row tiles
NC = 4          # 256-deep contraction chunks (DoubleRow pairs of 128)

F8 = ml_dtypes.float8_e4m3
BF = ml_dtypes.bfloat16

_COMPILED = {}


def _build_bass():
    import concourse.bacc as bacc
    import concourse.tile as tile
    from concourse import mybir

    f32 = mybir.dt.float32
    bf16 = mybir.dt.bfloat16
    fp8 = mybir.dt.float8e4
    AF = mybir.ActivationFunctionType
    alu = mybir.AluOpType
    DRm = mybir.MatmulPerfMode.DoubleRow

    nc = bacc.Bacc("TRN2", target_bir_lowering=False, debug=False)

    xt8_d = nc.dram_tensor("XT8", [128, NC, 2, S], fp8, kind="ExternalInput")
    xb_d = nc.dram_tensor("XB", [S, D], bf16, kind="ExternalInput")
    xb2_d = nc.dram_tensor("XB2", [128, NT], f32, kind="ExternalInput")
    xbs_d = nc.dram_tensor("XBS", [128, NT], f32, kind="ExternalInput")
    wqk_d = nc.dram_tensor("WQK8", [128, NC, 2, 128], fp8, kind="ExternalInput")
    wv_d = nc.dram_tensor("WV8", [128, NC, 2, K], fp8, kind="ExternalInput")
    wo_d = nc.dram_tensor("WOB8", [32, 2, D], fp8, kind="ExternalInput")
    wot_d = nc.dram_tensor("WOT8", [128, NC, 2, K], fp8, kind="ExternalInput")
    il_d = nc.dram_tensor("IL8", [32, 2, 2 * K + 1], fp8, kind="ExternalInput")
    bqk_d = nc.dram_tensor("BQK", [128, 1], f32, kind="ExternalInput")
    out_d = nc.dram_tensor("OUT", [S, D], bf16, kind="ExternalOutput")

    LN2 = 0.6931471805599453

    with tile.TileContext(nc) as tc:
        with (
            tc.tile_pool(name="consts", bufs=1) as consts,
            tc.tile_pool(name="bigx", bufs=1) as bigx,
            tc.tile_pool(name="proj", bufs=1) as proj,
            tc.tile_pool(name="expp", bufs=3) as expp,
            tc.tile_pool(name="ysb", bufs=8) as ysb,
            tc.tile_pool(name="y0p", bufs=4) as y0p,
            tc.tile_pool(name="scrp", bufs=1) as scrp,
            tc.tile_pool(name="outp", bufs=6) as outp,
            tc.tile_pool(name="work", bufs=4) as work,
            # PSUM rings: "pss" scores units (4 banks), "psu" uav
            # accumulators+psums (2 banks), "h" psqk/psv/psy-halves/psq3
            # (2 banks).
            tc.tile_pool(name="pss", bufs=2, space="PSUM") as psS,
            tc.tile_pool(name="psu", bufs=2, space="PSUM") as psU,
            tc.tile_pool(name="psh", bufs=2, space="PSUM") as psH,
        ):
            # ---- weights first (projections block on them), then XT8.
            wqk8 = consts.tile([128, NC, 2, 128], fp8)
            nc.sync.dma_start(out=wqk8, in_=wqk_d[:])
            bqk_col = consts.tile([128, 1], f32)
            nc.sync.dma_start(out=bqk_col, in_=bqk_d[:])
            xt8 = bigx.tile([128, NC, 2, S], fp8)
            for h in range(2):
                for c in range(NC):
                    nc.sync.dma_start(
                        out=xt8[:, c, :, h * 1024:(h + 1) * 1024],
                        in_=xt8_d[:, c, :, h * 1024:(h + 1) * 1024],
                    )
            wv8 = consts.tile([128, NC, 2, K], fp8)
            wob8 = consts.tile([32, 2, D], fp8)
            wot8 = consts.tile([128, NC, 2, K], fp8)
            il8 = consts.tile([32, 2, 2 * K + 1], fp8)
            xb2s = consts.tile([128, NT], f32)
            xbs_s = consts.tile([128, NT], f32)
            xb = bigx.tile([128, NT, D], bf16)
            xb_view = xb_d[:].rearrange("(t p) d -> p t d", p=128)

            ones16 = consts.tile([128, NT], bf16)
            nc.vector.memset(ones16, 1.0)
            nln2 = consts.tile([128, 1], f32)
            nc.vector.memset(nln2, -LN2)

            qkT8 = proj.tile([128, S], fp8)
            q8 = proj.tile([32, 2, S], fp8)
            k8 = proj.tile([32, 2, S], fp8)
            vh8 = proj.tile([128, NT, 2 * K], bf16)  # v | h=Wo@XB per tile
            avT8 = proj.tile([32, 2, S], fp8)
            uavT8 = proj.tile([K, S], fp8)
            rec_sb = work.tile([128, NT], f32)
            rec2_sb = work.tile([128, NT], f32)
            musum = work.tile([128, NT], f32)      # rowsum(y) via wdot trick
            csum = work.tile([128, NT], f32)       # recip * (uav . h)
            qsum = work.tile([128, NT], f32)       # recip^2 * |L.T uav|^2
            scr = scrp.tile([128, 2 * K], bf16)
            zs = scrp.tile([128, K], bf16)

            live = {}

            def emit_scores(tgt, units):
                """Scores + exp for query block tgt; units of 2 key-chunks."""
                if tgt not in live:
                    live[tgt] = expp.tile(
                        [128, NT, 512], bf16, tag="expT", name=f"expT{tgt}"
                    )
                et = live[tgt]
                sq = slice(tgt * 512, (tgt + 1) * 512)
                for u in units:
                    pss = psS.tile([128, 1024], f32, tag="pss", name=f"pss{tgt}_{u}")
                    for j in range(2):
                        sk = 2 * u + j
                        nc.tensor.matmul(
                            pss[:, j * 512:(j + 1) * 512],
                            k8[:, :, sk * 128:(sk + 1) * 128],
                            q8[:, :, sq],
                            start=True, stop=True, perf_mode=DRm,
                        )
                    nc.scalar.activation(
                        out=et[:, 2 * u:2 * u + 2, :].rearrange("p u q -> p (u q)"),
                        in_=pss,
                        func=AF.Exp, scale=0.125, bias=nln2,
                    )

            # ---- phase 1a: q/k projections + redistribution (compact
            # DVE TSP chain; v-projection deferred so the psH ring only
            # rotates psqk tiles here)
            for b in range(NB):
                sq = slice(b * 512, (b + 1) * 512)
                psqk = psH.tile([128, 512], f32, tag="h", name=f"psqk{b}")
                for c in range(NC):
                    nc.tensor.matmul(
                        psqk, wqk8[:, c], xt8[:, c, :, sq],
                        start=(c == 0), stop=(c == NC - 1), perf_mode=DRm,
                    )
                nc.vector.tensor_scalar(
                    out=qkT8[:, sq], in0=psqk, scalar1=bqk_col, scalar2=None,
                    op0=alu.add,
                )
                # redistribution littles (kappa = 32i+p); k8 gates score
                # emission so it lands per-block; q8 for blocks 2-3 is
                # phase-2-only and goes as one pair.
                nc.sync.dma_start(out=k8[:, 0, sq], in_=qkT8[64:96, sq])
                nc.sync.dma_start(out=k8[:, 1, sq], in_=qkT8[96:128, sq])
                if b <= 1:
                    nc.sync.dma_start(out=q8[:, 0, sq], in_=qkT8[0:32, sq])
                    nc.sync.dma_start(out=q8[:, 1, sq], in_=qkT8[32:64, sq])
                if b == 3:
                    sp = slice(1024, 2048)
                    nc.sync.dma_start(out=q8[:, 0, sp], in_=qkT8[0:32, sp])
                    nc.sync.dma_start(out=q8[:, 1, sp], in_=qkT8[32:64, sp])

                # keep block-0 exp units strictly ahead of block-1 in the
                # ACT queue; each unit is emitted after its k8 cols land.
                if b == 0:
                    emit_scores(0, [0, 1])
                elif b == 1:
                    emit_scores(0, [2, 3])
                elif b == 2:
                    emit_scores(0, [4, 5])
                else:
                    emit_scores(0, [6, 7])
                    emit_scores(1, range(0, 8))

            # deferred small weight loads (behind the phase-1a littles in
            # the SP queue; all are first needed well after them)
            nc.sync.dma_start(out=wv8, in_=wv_d[:])
            nc.sync.dma_start(out=wob8, in_=wo_d[:])
            nc.sync.dma_start(out=wot8, in_=wot_d[:])
            nc.sync.dma_start(out=il8, in_=il_d[:])
            nc.sync.dma_start(out=xb2s, in_=xb2_d[:])
            nc.sync.dma_start(out=xbs_s, in_=xbs_d[:])

            # ---- phase 1b: v projection (needed first at uav of block 0)
            for b in range(NB):
                psv = psH.tile([128, 4, 2 * K], f32, tag="h", name=f"psv{b}")
                for t in range(4):
                    sk = slice((4 * b + t) * 128, (4 * b + t + 1) * 128)
                    for c in range(NC):
                        nc.tensor.matmul(
                            psv[:, t, 0:K], xt8[:, c, :, sk], wv8[:, c],
                            start=(c == 0), stop=(c == NC - 1), perf_mode=DRm,
                        )
                    for c in range(NC):
                        nc.tensor.matmul(
                            psv[:, t, K:2 * K], xt8[:, c, :, sk], wot8[:, c],
                            start=(c == 0), stop=(c == NC - 1), perf_mode=DRm,
                        )
                nc.vector.tensor_copy(out=vh8[:, 4 * b:4 * b + 4, :], in_=psv)

            # XB loads ride the gpsimd SWDGE queue (pool is idle early),
            # gated on the LAST redistribution little (q8 block-3 pair) via
            # tiny fake writes so the transfers never block phase-1 DMAs.
            for g in range(8):
                nc.gpsimd.tensor_copy(
                    out=xb[0:1, 2 * g, 0:1], in_=q8[0:1, 0, S - 1:S]
                )
                nc.gpsimd.dma_start(
                    out=xb[:, 2 * g:2 * g + 2, :], in_=xb_view[:, 2 * g:2 * g + 2, :]
                )

            # ---- phase 2: uav + y + layernorm
            out_view = out_d[:].rearrange("(t p) d -> p t d", p=128)
            for b in range(NB):
                sq = slice(b * 512, (b + 1) * 512)
                expT = live.pop(b)
                psu = psU.tile([K, 512], f32, tag="psu", name=f"psuav{b}")
                for sk in range(NT):
                    nc.tensor.matmul(
                        psu, vh8[:, sk, 0:K], expT[:, sk, :],
                        start=(sk == 0), stop=(sk == NT - 1),
                    )
                if b < NB - 1:
                    nc.vector.tensor_copy(out=uavT8[:, sq], in_=psu)
                    nc.scalar.dma_start(out=avT8[:, 0, sq], in_=uavT8[0:32, sq])
                    nc.scalar.dma_start(out=avT8[:, 1, sq], in_=uavT8[32:64, sq])
                else:
                    nc.vector.tensor_copy(out=avT8[:, 0, sq], in_=psu[0:32, :])
                    nc.vector.tensor_copy(out=avT8[:, 1, sq], in_=psu[32:64, :])
                # softmax sums per query column via tiny PE accumulations
                psums = psU.tile([128, 4], f32, tag="psu", name=f"psums{b}")
                for j in range(4):
                    for sk in range(NT):
                        nc.tensor.matmul(
                            psums[:, j:j + 1],
                            expT[:, sk, j * 128:(j + 1) * 128],
                            ones16[:, 0:1],
                            start=(sk == 0), stop=(sk == NT - 1),
                        )
                bs = slice(4 * b, 4 * b + 4)
                nc.vector.reciprocal(out=rec_sb[:, bs], in_=psums)
                nc.vector.tensor_tensor(
                    out=rec2_sb[:, bs], in0=rec_sb[:, bs], in1=rec_sb[:, bs],
                    op=alu.mult,
                )

                if b + 2 < NB:
                    emit_scores(b + 2, range(8))

                last = b == NB - 1

                def emit_ti(t, act_assist):
                    yt = ysb.tile([128, D], bf16, tag="y", name=f"y{t}")
                    if act_assist:
                        y0 = y0p.tile([128, D], bf16, tag="y0", name=f"y0_{t}")
                    for j in range(2):
                        psy = psH.tile([128, 512], f32, tag="h", name=f"psy{t}_{j}")
                        nc.tensor.matmul(
                            psy,
                            avT8[:, :, t * 128:(t + 1) * 128],
                            wob8[:, :, j * 512:(j + 1) * 512],
                            start=True, stop=True, perf_mode=DRm,
                        )
                        if act_assist:
                            # ACT is past its exp stream here: scale on ACT,
                            # residual add on DVE at 2x (all-SBUF bf16)
                            nc.scalar.mul(
                                out=y0[:, j * 512:(j + 1) * 512], in_=psy,
                                mul=rec_sb[:, t:t + 1],
                            )
                        else:
                            nc.vector.scalar_tensor_tensor(
                                out=yt[:, j * 512:(j + 1) * 512],
                                in0=psy, scalar=rec_sb[:, t:t + 1],
                                in1=xb[:, t, j * 512:(j + 1) * 512],
                                op0=alu.mult, op1=alu.add,
                            )
                    if act_assist:
                        nc.vector.tensor_tensor(
                            out=yt, in0=y0, in1=xb[:, t, :], op=alu.add,
                        )
                    # LN pieces: uav-nat | z = L.T uav | wdot (h = Wo@XB
                    # precomputed in phase 1b; one PSUM input per DVE op)
                    psq3 = psH.tile([128, 2 * K + 1], f32, tag="h",
                                    name=f"psq3{t}")
                    tsl = slice(t * 128, (t + 1) * 128)
                    nc.tensor.matmul(
                        psq3, avT8[:, :, tsl], il8,
                        start=True, stop=True, perf_mode=DRm,
                    )
                    nc.vector.scalar_tensor_tensor(
                        out=scr[:, 0:K], in0=psq3[:, 0:K],
                        scalar=rec_sb[:, t:t + 1], in1=vh8[:, t, K:2 * K],
                        op0=alu.mult, op1=alu.mult,
                        accum_out=csum[:, t:t + 1],
                    )
                    if act_assist:
                        nc.scalar.mul(
                            out=zs, in_=psq3[:, K:2 * K], mul=rec_sb[:, t:t + 1]
                        )
                    else:
                        nc.vector.tensor_scalar(
                            out=zs, in0=psq3[:, K:2 * K],
                            scalar1=rec_sb[:, t:t + 1], scalar2=None, op0=alu.mult,
                        )
                    nc.vector.scalar_tensor_tensor(
                        out=scr[:, K:2 * K], in0=zs, scalar=1.0, in1=zs,
                        op0=alu.mult, op1=alu.mult,
                        accum_out=qsum[:, t:t + 1],
                    )
                    # rowsum(y) = xbs + rec*wdot  (no accumulator needed)
                    nc.vector.scalar_tensor_tensor(
                        out=musum[:, t:t + 1], in0=psq3[:, 2 * K:2 * K + 1],
                        scalar=rec_sb[:, t:t + 1], in1=xbs_s[:, t:t + 1],
                        op0=alu.mult, op1=alu.add,
                    )
                    live[f"y{t}"] = yt

                def emit_group(t0, n, eng, neng):
                    ts = slice(t0, t0 + n)
                    mu = work.tile([128, n], f32, tag="mu")
                    eng.tensor_scalar(
                        out=mu, in0=musum[:, ts], scalar1=1.0 / D, scalar2=None,
                        op0=alu.mult,
                    )
                    # ve = (xb2 + 2*csum + qsum)/D + EPS - mu^2
                    ve = work.tile([128, n], f32, tag="ve")
                    eng.tensor_tensor(out=ve, in0=csum[:, ts], in1=csum[:, ts],
                                      op=alu.add)
                    eng.tensor_tensor(out=ve, in0=ve, in1=qsum[:, ts], op=alu.add)
                    eng.tensor_tensor(out=ve, in0=ve, in1=xb2s[:, ts], op=alu.add)
                    eng.tensor_scalar(
                        out=ve, in0=ve, scalar1=1.0 / D, scalar2=EPS,
                        op0=alu.mult, op1=alu.add,
                    )
                    mu2 = work.tile([128, n], f32, tag="mu2")
                    eng.tensor_tensor(out=mu2, in0=mu, in1=mu, op=alu.mult)
                    eng.tensor_tensor(out=ve, in0=ve, in1=mu2, op=alu.subtract)
                    # rstd = rsqrt(ve) via multiply-only Newton (w ~ [0.7, 1.4])
                    rstd = work.tile([128, n], f32, tag="rstd")
                    eng.tensor_scalar(
                        out=rstd, in0=ve, scalar1=-0.5, scalar2=1.5,
                        op0=alu.mult, op1=alu.add,
                    )
                    for _ in range(2):
                        na = work.tile([128, n], f32, tag="na")
                        eng.tensor_tensor(out=na, in0=rstd, in1=rstd, op=alu.mult)
                        eng.tensor_tensor(out=na, in0=na, in1=ve, op=alu.mult)
                        eng.tensor_scalar(
                            out=na, in0=na, scalar1=-0.5, scalar2=1.5,
                            op0=alu.mult, op1=alu.add,
                        )
                        rstd2 = work.tile([128, n], f32, tag="rstd")
                        eng.tensor_tensor(out=rstd2, in0=rstd, in1=na, op=alu.mult)
                        rstd = rstd2
                    for t in range(t0, t0 + n):
                        yt = live.pop(f"y{t}")
                        ot = outp.tile([128, D], bf16, tag="o")
                        neng.tensor_scalar(
                            out=ot, in0=yt,
                            scalar1=mu[:, t - t0:t - t0 + 1],
                            scalar2=rstd[:, t - t0:t - t0 + 1],
                            op0=alu.subtract, op1=alu.mult,
                        )
                        nc.sync.dma_start(out=out_view[:, t, :], in_=ot)

                if b <= 1:
                    for t in range(4 * b, 4 * b + 4):
                        emit_ti(t, False)
                    emit_group(4 * b, 4, nc.gpsimd, nc.gpsimd)
                elif b == 2:
                    for t in range(8, 12):
                        emit_ti(t, True)
                    emit_group(8, 4, nc.gpsimd, nc.gpsimd)
                else:
                    # pair-interleaved drain on DVE+ACT
                    emit_ti(12, True)
                    emit_ti(13, True)
                    emit_group(12, 2, nc.vector, nc.vector)
                    emit_ti(14, True)
                    emit_ti(15, True)
                    emit_group(14, 2, nc.vector, nc.vector)

    nc.compile()
    return nc


def _get_compiled():
    if "nc" not in _COMPILED:
        _COMPILED["nc"] = _build_bass()
    return _COMPILED["nc"]


def _prep_weights(Wq, bq, Wk, bk, Wv, bv, Wo, bo):
    """Host-side folding + DR packing. d = 2*(128c+p)+i for contractions."""
    bo_f = bo + bv @ Wo
    # r(I + Wv@Wo) = bo'  ->  the single input shift serving proj + residual
    r = np.linalg.solve((np.eye(D, dtype=np.float64) + Wv.astype(np.float64)
                         @ Wo.astype(np.float64)).T, bo_f.astype(np.float64))
    r = r.astype(np.float32)
    bq_f = bq - r @ Wq

    wqk = np.concatenate([Wq, Wk], axis=1)  # [D, 128]
    p = np.arange(128)
    wqk8 = np.zeros((128, NC, 2, 128), dtype=F8)
    wv8 = np.zeros((128, NC, 2, K), dtype=F8)
    wot8 = np.zeros((128, NC, 2, K), dtype=F8)
    WoT = np.ascontiguousarray(Wo.T)  # [D, K]
    for c in range(NC):
        for i in range(2):
            ds = 2 * (128 * c + p) + i
            wqk8[:, c, i, :] = wqk[ds, :].astype(F8)
            wv8[:, c, i, :] = Wv[ds, :].astype(F8)
            wot8[:, c, i, :] = WoT[ds, :].astype(F8)
    # kappa = 32i + p: Wo rows [0:32] -> plane 0, [32:64] -> plane 1
    wob8 = np.zeros((32, 2, D), dtype=F8)
    wob8[:, 0, :] = Wo[0:32, :].astype(F8)
    wob8[:, 1, :] = Wo[32:64, :].astype(F8)
    # il8 = [I | L | worow] for the uav-nat, z = L.T uav, wdot matmuls
    G = (Wo @ Wo.T).astype(np.float64)
    L = np.linalg.cholesky(G + 1e-9 * np.eye(K))
    il = np.zeros((K, 2 * K + 1), dtype=np.float32)
    il[:, 0:K] = np.eye(K, dtype=np.float32)
    il[:, K:2 * K] = L.astype(np.float32)
    il[:, 2 * K] = Wo.sum(axis=1)
    il8 = np.zeros((32, 2, 2 * K + 1), dtype=F8)
    il8[:, 0, :] = il[0:32, :].astype(F8)
    il8[:, 1, :] = il[32:64, :].astype(F8)
    bqk = np.zeros((128, 1), dtype=np.float32)
    bqk[0:K, 0] = bq_f
    return wqk8, wv8, wob8, wot8, il8, bqk, r


def kernel(X, Wq, bq, Wk, bk, Wv, bv, Wo, bo, gamma, beta):
    from concourse.bass_utils import run_bass_kernel_spmd

    X = np.ascontiguousarray(np.asarray(X, dtype=np.float32))
    Wq, bq = np.asarray(Wq, np.float32), np.asarray(bq, np.float32)
    Wk, bk = np.asarray(Wk, np.float32), np.asarray(bk, np.float32)
    Wv, bv = np.asarray(Wv, np.float32), np.asarray(bv, np.float32)
    Wo, bo = np.asarray(Wo, np.float32), np.asarray(bo, np.float32)
    gamma_np = np.asarray(gamma, dtype=np.float32)
    beta_np = np.asarray(beta, dtype=np.float32)

    wqk8, wv8, wob8, wot8, il8, bqk, r = _prep_weights(
        Wq, bq, Wk, bk, Wv, bv, Wo, bo
    )

    nc = _get_compiled()
    in_maps = []
    for bi in range(B):
        Xr = X[bi] + r
        X8 = Xr.astype(F8)
        # XT8[p, c, i, s] = X8[s, 256c + 2p + i]
        xt8 = np.ascontiguousarray(
            X8.reshape(S, NC, 128, 2).transpose(2, 1, 3, 0)
        )
        xbi = Xr.astype(BF)
        xbf = xbi.astype(np.float32)
        xb2 = (xbf ** 2).sum(axis=1)  # [S]
        xb2t = np.ascontiguousarray(xb2.reshape(NT, 128).T)  # [128, NT]
        xbs = xbf.sum(axis=1)
        xbst = np.ascontiguousarray(xbs.reshape(NT, 128).T)
        in_maps.append({
            "XT8": xt8, "XB": xbi, "XB2": xb2t, "XBS": xbst,
            "WQK8": wqk8, "WV8": wv8, "WOB8": wob8, "WOT8": wot8,
            "IL8": il8, "BQK": bqk,
        })
    res = run_bass_kernel_spmd(nc, in_maps, core_ids=list(range(B)))
    out = np.stack(
        [np.asarray(res.results[i]["OUT"], dtype=np.float32) for i in range(B)],
        axis=0,
    )
    if not (np.all(gamma_np == 1.0) and np.all(beta_np == 0.0)):
        out = out * gamma_np + beta_np
    return out.astype(np.float32)


# revision 40
# speedup vs baseline: 1.0589x; 1.0296x over previous
"""Trainium2 Bass kernel: batched single-head attention + residual + layernorm.

Reference (per batch element b of 8, one NeuronCore each — data-parallel):
    q = X@Wq+bq; k = X@Wk+bk; v = X@Wv+bv          [S=2048, K=64]
    attn = softmax(q @ k.T / 8, axis=-1)            [S, S]
    y = X + (attn @ v) @ Wo + bo                    [S, D=1024]
    out = layernorm(y) * gamma + beta

v4 design (fp8 DoubleRow everywhere it pays + matmul-based LN stats):
  Host algebra: bk drops out of softmax (constant along keys); bo'=bo+bv@Wo;
  r solves r(I + Wv@Wo) = bo' so ONE shifted input X+r serves both the
  projections (q bias corrected by -r@Wq; k/v shifts cancel exactly) and the
  residual XB = bf16(X+r). Host uploads XT8 = fp8(X+r) pre-transposed in
  DoubleRow pair layout [128,c,i,s] (d = 2(128c+p)+i), XB, DR-packed fp8
  weights, and rowsum(XB^2). OUT returns bf16, upcast on host.

  Device per core:
  1. q/k projection: 4 fp8 DR matmuls -> psqk. Block 0 is redistributed into
     q8/k8 [32,2,S] DR layouts by 4 direct DVE ops (jumpstart, hidden in the
     startup bubble); blocks 1-3 go via one DVE cast to qkT8 fp8 + tiny
     SBUF->SBUF DMAs. kappa = 32i+p.
  2. v projection: DR matmuls with lhsT=XT8 s-slice emit v in natural [s,K]
     layout; DVE-copied to bf16 v8 with a ones column (softmax sums).
  3. scores: one DR matmul per 128-key chunk (contraction 2x32); exp on ACT
     in [128,1024] two-chunk units, bias -ln2 (fp8 range guard), bf16 expT.
     Emission order keeps block-0 units ahead of block-1 in the ACT queue.
  4. uav: bf16 matmuls accumulate the attention numerator AND softmax sums
     (ones column) in one PSUM group; uavT8 = DVE fp8 cast (division is
     deferred into the residual pass); sums row ACT-copied to SBUF,
     PE-transposed to per-query columns, reciprocal + recip^2 on DVE.
  5. y: Wo matmul as fp8 DR from avT8; residual + softmax division fused in
     one DVE scalar_tensor_tensor per psy half, accum = rowsum(y) -> mean.
  6. Variance WITHOUT a quadratic pass over y: rowsum(y^2) = rowsum(XB^2)
     [host] + 2 recip (uav . h) + recip^2 (uav . G uav), where h = Wo@XB
     comes from tiny DR matmuls against XT8, G = Wo@Wo.T = L L.T (host
     Cholesky), z = L.T uav from one tiny DR matmul. The two dots are 192ns
     DVE scalar_tensor_tensor ops over [128,64] PSUM with recip/recip^2 in
     the scalar slot. Newton rsqrt + variance combine on GPSIMD (pool);
     normalize on DVE at 4x (bf16 SBUF); last block runs stats on DVE to
     shorten the drain tail. Output bf16.
"""

import numpy as np
import ml_dtypes

B = 8
S = 2048
D = 1024
K = 64
EPS = 1e-5
NB = 4          # 512-query blocks
NT = 16         # 128-row tiles
NC = 4          # 256-deep contraction chunks (DoubleRow pairs of 128)

F8 = ml_dtypes.float8_e4m3
BF = ml_dtypes.bfloat16

_COMPILED = {}


def _build_bass():
    import concourse.bacc as bacc
    import concourse.tile as tile
    from concourse import mybir

    f32 = mybir.dt.float32
    bf16 = mybir.dt.bfloat16
    fp8 = mybir.dt.float8e4
    AF = mybir.ActivationFunctionType
    alu = mybir.AluOpType
    DRm = mybir.MatmulPerfMode.DoubleRow

    nc = bacc.Bacc("TRN2", target_bir_lowering=False, debug=False)

    xt8_d = nc.dram_tensor("XT8", [128, NC, 2, S], fp8, kind="ExternalInput")
    xb_d = nc.dram_tensor("XB", [S, D], bf16, kind="ExternalInput")
    xb2_d = nc.dram_tensor("XB2", [128, NT], f32, kind="ExternalInput")
    xbs_d = nc.dram_tensor("XBS", [128, NT], f32, kind="ExternalInput")
    wqk_d = nc.dram_tensor("WQK8", [128, NC, 2, 128], fp8, kind="ExternalInput")
    wv_d = nc.dram_tensor("WV8", [128, NC, 2, K], fp8, kind="ExternalInput")
    wo_d = nc.dram_tensor("WOB8", [32, 2, D], fp8, kind="ExternalInput")
    wot_d = nc.dram_tensor("WOT8", [128, NC, 2, K], fp8, kind="ExternalInput")
    il_d = nc.dram_tensor("IL8", [32, 2, 2 * K + 1], fp8, kind="ExternalInput")
    bqk_d = nc.dram_tensor("BQK", [128, 1], f32, kind="ExternalInput")
    out_d = nc.dram_tensor("OUT", [S, D], bf16, kind="ExternalOutput")

    LN2 = 0.6931471805599453

    with tile.TileContext(nc) as tc:
        with (
            tc.tile_pool(name="consts", bufs=1) as consts,
            tc.tile_pool(name="bigx", bufs=1) as bigx,
            tc.tile_pool(name="proj", bufs=1) as proj,
            tc.tile_pool(name="expp", bufs=3) as expp,
            tc.tile_pool(name="ysb", bufs=8) as ysb,
            tc.tile_pool(name="y0p", bufs=4) as y0p,
            tc.tile_pool(name="scrp", bufs=1) as scrp,
            tc.tile_pool(name="outp", bufs=6) as outp,
            tc.tile_pool(name="work", bufs=4) as work,
            # PSUM rings: "pss" scores units (4 banks), "psu" uav
            # accumulators+psums (2 banks), "h" psqk/psv/psy-halves/psq3
            # (2 banks).
            tc.tile_pool(name="pss", bufs=2, space="PSUM") as psS,
            tc.tile_pool(name="psu", bufs=2, space="PSUM") as psU,
            tc.tile_pool(name="psh", bufs=2, space="PSUM") as psH,
        ):
            # ---- weights first (projections block on them), then XT8.
            wqk8 = consts.tile([128, NC, 2, 128], fp8)
            nc.sync.dma_start(out=wqk8, in_=wqk_d[:])
            bqk_col = consts.tile([128, 1], f32)
            nc.sync.dma_start(out=bqk_col, in_=bqk_d[:])
            xt8 = bigx.tile([128, NC, 2, S], fp8)
            for h in range(2):
                for c in range(NC):
                    nc.sync.dma_start(
                        out=xt8[:, c, :, h * 1024:(h + 1) * 1024],
                        in_=xt8_d[:, c, :, h * 1024:(h + 1) * 1024],
                    )
            wv8 = consts.tile([128, NC, 2, K], fp8)
            wob8 = consts.tile([32, 2, D], fp8)
            wot8 = consts.tile([128, NC, 2, K], fp8)
            il8 = consts.tile([32, 2, 2 * K + 1], fp8)
            xb2s = consts.tile([128, NT], f32)
            xbs_s = consts.tile([128, NT], f32)
            xb = bigx.tile([128, NT, D], bf16)
            xb_view = xb_d[:].rearrange("(t p) d -> p t d", p=128)

            ones16 = consts.tile([128, NT], bf16)
            nc.vector.memset(ones16, 1.0)
            nln2 = consts.tile([128, 1], f32)
            nc.vector.memset(nln2, -LN2)

            qkT8 = proj.tile([128, S], fp8)
            q8 = proj.tile([32, 2, S], fp8)
            k8 = proj.tile([32, 2, S], fp8)
            vh8 = proj.tile([128, NT, 2 * K], bf16)  # v | h=Wo@XB per tile
            avT8 = proj.tile([32, 2, S], fp8)
            uavT8 = proj.tile([K, S], fp8)
            rec_sb = work.tile([128, NT], f32)
            rec2_sb = work.tile([128, NT], f32)
            musum = work.tile([128, NT], f32)      # rowsum(y) via wdot trick
            csum = work.tile([128, NT], f32)       # recip * (uav . h)
            qsum = work.tile([128, NT], f32)       # recip^2 * |L.T uav|^2
            scr = scrp.tile([128, 2 * K], bf16)
            zs = scrp.tile([128, K], bf16)

            live = {}

            def emit_scores(tgt, units):
                """Scores + exp for query block tgt; units of 2 key-chunks."""
                if tgt not in live:
                    live[tgt] = expp.tile(
                        [128, NT, 512], bf16, tag="expT", name=f"expT{tgt}"
                    )
                et = live[tgt]
                sq = slice(tgt * 512, (tgt + 1) * 512)
                for u in units:
                    pss = psS.tile([128, 1024], f32, tag="pss", name=f"pss{tgt}_{u}")
                    for j in range(2):
                        sk = 2 * u + j
                        nc.tensor.matmul(
                            pss[:, j * 512:(j + 1) * 512],
                            k8[:, :, sk * 128:(sk + 1) * 128],
                            q8[:, :, sq],
                            start=True, stop=True, perf_mode=DRm,
                        )
                    nc.scalar.activation(
                        out=et[:, 2 * u:2 * u + 2, :].rearrange("p u q -> p (u q)"),
                        in_=pss,
                        func=AF.Exp, scale=0.125, bias=nln2,
                    )

            # ---- phase 1a: q/k projections + redistribution (compact
            # DVE TSP chain; v-projection deferred so the psH ring only
            # rotates psqk tiles here)
            for b in range(NB):
                sq = slice(b * 512, (b + 1) * 512)
                psqk = psH.tile([128, 512], f32, tag="h", name=f"psqk{b}")
                for c in range(NC):
                    nc.tensor.matmul(
                        psqk, wqk8[:, c], xt8[:, c, :, sq],
                        start=(c == 0), stop=(c == NC - 1), perf_mode=DRm,
                    )
                if b == 0:
                    # jumpstart block 0 on idle DVE: skip qkT8 + littles
                    nc.vector.tensor_scalar(
                        out=q8[:, 0, sq], in0=psqk[0:32], scalar1=bqk_col[0:32],
                        scalar2=None, op0=alu.add,
                    )
                    nc.vector.tensor_scalar(
                        out=q8[:, 1, sq], in0=psqk[32:64], scalar1=bqk_col[32:64],
                        scalar2=None, op0=alu.add,
                    )
                    nc.vector.tensor_copy(out=k8[:, 0, sq], in_=psqk[64:96])
                    nc.vector.tensor_copy(out=k8[:, 1, sq], in_=psqk[96:128])
                else:
                    nc.vector.tensor_scalar(
                        out=qkT8[:, sq], in0=psqk, scalar1=bqk_col, scalar2=None,
                        op0=alu.add,
                    )
                    # redistribution littles (kappa = 32i+p)
                    nc.sync.dma_start(out=k8[:, 0, sq], in_=qkT8[64:96, sq])
                    nc.sync.dma_start(out=k8[:, 1, sq], in_=qkT8[96:128, sq])
                if b == 1:
                    nc.sync.dma_start(out=q8[:, 0, sq], in_=qkT8[0:32, sq])
                    nc.sync.dma_start(out=q8[:, 1, sq], in_=qkT8[32:64, sq])
                if b == 3:
                    sp = slice(1024, 2048)
                    nc.sync.dma_start(out=q8[:, 0, sp], in_=qkT8[0:32, sp])
                    nc.sync.dma_start(out=q8[:, 1, sp], in_=qkT8[32:64, sp])

                # keep block-0 exp units strictly ahead of block-1 in the
                # ACT queue; each unit is emitted after its k8 cols land.
                if b == 0:
                    emit_scores(0, [0, 1])
                elif b == 1:
                    emit_scores(0, [2, 3])
                elif b == 2:
                    emit_scores(0, [4, 5])
                else:
                    emit_scores(0, [6, 7])
                    emit_scores(1, range(0, 8))

            # deferred small weight loads (behind the phase-1a littles in
            # the SP queue; all are first needed well after them)
            nc.sync.dma_start(out=wv8, in_=wv_d[:])
            nc.sync.dma_start(out=wob8, in_=wo_d[:])
            nc.sync.dma_start(out=wot8, in_=wot_d[:])
            nc.sync.dma_start(out=il8, in_=il_d[:])
            nc.sync.dma_start(out=xb2s, in_=xb2_d[:])
            nc.sync.dma_start(out=xbs_s, in_=xbs_d[:])

            # ---- phase 1b: v projection (needed first at uav of block 0)
            for b in range(NB):
                psv = psH.tile([128, 4, 2 * K], f32, tag="h", name=f"psv{b}")
                for t in range(4):
                    sk = slice((4 * b + t) * 128, (4 * b + t + 1) * 128)
                    for c in range(NC):
                        nc.tensor.matmul(
                            psv[:, t, 0:K], xt8[:, c, :, sk], wv8[:, c],
                            start=(c == 0), stop=(c == NC - 1), perf_mode=DRm,
                        )
                    for c in range(NC):
                        nc.tensor.matmul(
                            psv[:, t, K:2 * K], xt8[:, c, :, sk], wot8[:, c],
                            start=(c == 0), stop=(c == NC - 1), perf_mode=DRm,
                        )
                nc.vector.tensor_copy(out=vh8[:, 4 * b:4 * b + 4, :], in_=psv)

            # XB loads ride the gpsimd SWDGE queue (pool is idle early),
            # gated on the LAST redistribution little (q8 block-3 pair) via
            # tiny fake writes so the transfers never block phase-1 DMAs.
            for g in range(8):
                nc.gpsimd.tensor_copy(
                    out=xb[0:1, 2 * g, 0:1], in_=q8[0:1, 0, S - 1:S]
                )
                nc.gpsimd.dma_start(
                    out=xb[:, 2 * g:2 * g + 2, :], in_=xb_view[:, 2 * g:2 * g + 2, :]
                )

            # ---- phase 2: uav + y + layernorm
            out_view = out_d[:].rearrange("(t p) d -> p t d", p=128)
            for b in range(NB):
                sq = slice(b * 512, (b + 1) * 512)
                expT = live.pop(b)
                psu = psU.tile([K, 512], f32, tag="psu", name=f"psuav{b}")
                for sk in range(NT):
                    nc.tensor.matmul(
                        psu, vh8[:, sk, 0:K], expT[:, sk, :],
                        start=(sk == 0), stop=(sk == NT - 1),
                    )
                if b < NB - 1:
                    nc.vector.tensor_copy(out=uavT8[:, sq], in_=psu)
                    nc.scalar.dma_start(out=avT8[:, 0, sq], in_=uavT8[0:32, sq])
                    nc.scalar.dma_start(out=avT8[:, 1, sq], in_=uavT8[32:64, sq])
                else:
                    nc.vector.tensor_copy(out=avT8[:, 0, sq], in_=psu[0:32, :])
                    nc.vector.tensor_copy(out=avT8[:, 1, sq], in_=psu[32:64, :])
                # softmax sums per query column via tiny PE accumulations
                psums = psU.tile([128, 4], f32, tag="psu", name=f"psums{b}")
                for j in range(4):
                    for sk in range(NT):
                        nc.tensor.matmul(
                            psums[:, j:j + 1],
                            expT[:, sk, j * 128:(j + 1) * 128],
                            ones16[:, 0:1],
                            start=(sk == 0), stop=(sk == NT - 1),
                        )
                bs = slice(4 * b, 4 * b + 4)
                nc.vector.reciprocal(out=rec_sb[:, bs], in_=psums)
                nc.vector.tensor_tensor(
                    out=rec2_sb[:, bs], in0=rec_sb[:, bs], in1=rec_sb[:, bs],
                    op=alu.mult,
                )

                if b + 2 < NB:
                    emit_scores(b + 2, range(8))

                last = b == NB - 1

                def emit_ti(t, act_assist):
                    yt = ysb.tile([128, D], bf16, tag="y", name=f"y{t}")
                    if act_assist:
                        y0 = y0p.tile([128, D], bf16, tag="y0", name=f"y0_{t}")
                    for j in range(2):
                        psy = psH.tile([128, 512], f32, tag="h", name=f"psy{t}_{j}")
                        nc.tensor.matmul(
                            psy,
                            avT8[:, :, t * 128:(t + 1) * 128],
                            wob8[:, :, j * 512:(j + 1) * 512],
                            start=True, stop=True, perf_mode=DRm,
                        )
                        if act_assist:
                            # ACT is past its exp stream here: scale on ACT,
                            # residual add on DVE at 2x (all-SBUF bf16)
                            nc.scalar.mul(
                                out=y0[:, j * 512:(j + 1) * 512], in_=psy,
                                mul=rec_sb[:, t:t + 1],
                            )
                        else:
                            nc.vector.scalar_tensor_tensor(
                                out=yt[:, j * 512:(j + 1) * 512],
                                in0=psy, scalar=rec_sb[:, t:t + 1],
                                in1=xb[:, t, j * 512:(j + 1) * 512],
                                op0=alu.mult, op1=alu.add,
                            )
                    if act_assist:
                        nc.vector.tensor_tensor(
                            out=yt, in0=y0, in1=xb[:, t, :], op=alu.add,
                        )
                    # LN pieces: uav-nat | z = L.T uav | wdot (h = Wo@XB
                    # precomputed in phase 1b; one PSUM input per DVE op)
                    psq3 = psH.tile([128, 2 * K + 1], f32, tag="h",
                                    name=f"psq3{t}")
                    tsl = slice(t * 128, (t + 1) * 128)
                    nc.tensor.matmul(
                        psq3, avT8[:, :, tsl], il8,
                        start=True, stop=True, perf_mode=DRm,
                    )
                    nc.vector.scalar_tensor_tensor(
                        out=scr[:, 0:K], in0=psq3[:, 0:K],
                        scalar=rec_sb[:, t:t + 1], in1=vh8[:, t, K:2 * K],
                        op0=alu.mult, op1=alu.mult,
                        accum_out=csum[:, t:t + 1],
                    )
                    if act_assist:
                        nc.scalar.mul(
                            out=zs, in_=psq3[:, K:2 * K], mul=rec_sb[:, t:t + 1]
                        )
                    else:
                        nc.vector.tensor_scalar(
                            out=zs, in0=psq3[:, K:2 * K],
                            scalar1=rec_sb[:, t:t + 1], scalar2=None, op0=alu.mult,
                        )
                    nc.vector.scalar_tensor_tensor(
                        out=scr[:, K:2 * K], in0=zs, scalar=1.0, in1=zs,
                        op0=alu.mult, op1=alu.mult,
                        accum_out=qsum[:, t:t + 1],
                    )
                    # rowsum(y) = xbs + rec*wdot  (no accumulator needed)
                    nc.vector.scalar_tensor_tensor(
                        out=musum[:, t:t + 1], in0=psq3[:, 2 * K:2 * K + 1],
                        scalar=rec_sb[:, t:t + 1], in1=xbs_s[:, t:t + 1],
                        op0=alu.mult, op1=alu.add,
                    )
                    live[f"y{t}"] = yt

                def emit_group(t0, n, eng, neng):
                    ts = slice(t0, t0 + n)
                    mu = work.tile([128, n], f32, tag="mu")
                    eng.tensor_scalar(
                        out=mu, in0=musum[:, ts], scalar1=1.0 / D, scalar2=None,
                        op0=alu.mult,
                    )
                    # ve = (xb2 + 2*csum + qsum)/D + EPS - mu^2
                    ve = work.tile([128, n], f32, tag="ve")
                    eng.tensor_tensor(out=ve, in0=csum[:, ts], in1=csum[:, ts],
                                      op=alu.add)
                    eng.tensor_tensor(out=ve, in0=ve, in1=qsum[:, ts], op=alu.add)
                    eng.tensor_tensor(out=ve, in0=ve, in1=xb2s[:, ts], op=alu.add)
                    eng.tensor_scalar(
                        out=ve, in0=ve, scalar1=1.0 / D, scalar2=EPS,
                        op0=alu.mult, op1=alu.add,
                    )
                    mu2 = work.tile([128, n], f32, tag="mu2")
                    eng.tensor_tensor(out=mu2, in0=mu, in1=mu, op=alu.mult)
                    eng.tensor_tensor(out=ve, in0=ve, in1=mu2, op=alu.subtract)
                    # rstd = rsqrt(ve) via multiply-only Newton (w ~ [0.7, 1.4])
                    rstd = work.tile([128, n], f32, tag="rstd")
                    eng.tensor_scalar(
                        out=rstd, in0=ve, scalar1=-0.5, scalar2=1.5,
                        op0=alu.mult, op1=alu.add,
                    )
                    for _ in range(2):
                        na = work.tile([128, n], f32, tag="na")
                        eng.tensor_tensor(out=na, in0=rstd, in1=rstd, op=alu.mult)
                        eng.tensor_tensor(out=na, in0=na, in1=ve, op=alu.mult)
                        eng.tensor_scalar(
                            out=na, in0=na, scalar1=-0.5, scalar2=1.5,
                            op0=alu.mult, op1=alu.add,
                        )
                        rstd2 = work.tile([128, n], f32, tag="rstd")
                        eng.tensor_tensor(out=rstd2, in0=rstd, in1=na, op=alu.mult)
                        rstd = rstd2
                    for t in range(t0, t0 + n):
                        yt = live.pop(f"y{t}")
                        ot = outp.tile([128, D], bf16, tag="o")
                        neng.tensor_scalar(
                            out=ot, in0=yt,
                            scalar1=mu[:, t - t0:t - t0 + 1],
                            scalar2=rstd[:, t - t0:t - t0 + 1],
                            op0=alu.subtract, op1=alu.mult,
                        )
                        nc.sync.dma_start(out=out_view[:, t, :], in_=ot)

                if b <= 1:
                    for t in range(4 * b, 4 * b + 4):
                        emit_ti(t, False)
                    emit_group(4 * b, 4, nc.gpsimd, nc.gpsimd)
                elif b == 2:
                    for t in range(8, 12):
                        emit_ti(t, True)
                    emit_group(8, 4, nc.gpsimd, nc.gpsimd)
                else:
                    # pair-interleaved drain on DVE+ACT
                    emit_ti(12, True)
                    emit_ti(13, True)
                    emit_group(12, 2, nc.vector, nc.vector)
                    emit_ti(14, True)
                    emit_ti(15, True)
                    emit_group(14, 2, nc.vector, nc.vector)

    nc.compile()
    return nc


def _get_compiled():
    if "nc" not in _COMPILED:
        _COMPILED["nc"] = _build_bass()
    return _COMPILED["nc"]


def _prep_weights(Wq, bq, Wk, bk, Wv, bv, Wo, bo):
    """Host-side folding + DR packing. d = 2*(128c+p)+i for contractions."""
    bo_f = bo + bv @ Wo
    # r(I + Wv@Wo) = bo'  ->  the single input shift serving proj + residual
    r = np.linalg.solve((np.eye(D, dtype=np.float64) + Wv.astype(np.float64)
                         @ Wo.astype(np.float64)).T, bo_f.astype(np.float64))
    r = r.astype(np.float32)
    bq_f = bq - r @ Wq

    wqk = np.concatenate([Wq, Wk], axis=1)  # [D, 128]
    p = np.arange(128)
    wqk8 = np.zeros((128, NC, 2, 128), dtype=F8)
    wv8 = np.zeros((128, NC, 2, K), dtype=F8)
    wot8 = np.zeros((128, NC, 2, K), dtype=F8)
    WoT = np.ascontiguousarray(Wo.T)  # [D, K]
    for c in range(NC):
        for i in range(2):
            ds = 2 * (128 * c + p) + i
            wqk8[:, c, i, :] = wqk[ds, :].astype(F8)
            wv8[:, c, i, :] = Wv[ds, :].astype(F8)
            wot8[:, c, i, :] = WoT[ds, :].astype(F8)
    # kappa = 32i + p: Wo rows [0:32] -> plane 0, [32:64] -> plane 1
    wob8 = np.zeros((32, 2, D), dtype=F8)
    wob8[:, 0, :] = Wo[0:32, :].astype(F8)
    wob8[:, 1, :] = Wo[32:64, :].astype(F8)
    # il8 = [I | L | worow] for the uav-nat, z = L.T uav, wdot matmuls
    G = (Wo @ Wo.T).astype(np.float64)
    L = np.linalg.cholesky(G + 1e-9 * np.eye(K))
    il = np.zeros((K, 2 * K + 1), dtype=np.float32)
    il[:, 0:K] = np.eye(K, dtype=np.float32)
    il[:, K:2 * K] = L.astype(np.float32)
    il[:, 2 * K] = Wo.sum(axis=1)
    il8 = np.zeros((32, 2, 2 * K + 1), dtype=F8)
    il8[:, 0, :] = il[0:32, :].astype(F8)
    il8[:, 1, :] = il[32:64, :].astype(F8)
    bqk = np.zeros((128, 1), dtype=np.float32)
    bqk[0:K, 0] = bq_f
    return wqk8, wv8, wob8, wot8, il8, bqk, r


def kernel(X, Wq, bq, Wk, bk, Wv, bv, Wo, bo, gamma, beta):
    from concourse.bass_utils import run_bass_kernel_spmd

    X = np.ascontiguousarray(np.asarray(X, dtype=np.float32))
    Wq, bq = np.asarray(Wq, np.float32), np.asarray(bq, np.float32)
    Wk, bk = np.asarray(Wk, np.float32), np.asarray(bk, np.float32)
    Wv, bv = np.asarray(Wv, np.float32), np.asarray(bv, np.float32)
    Wo, bo = np.asarray(Wo, np.float32), np.asarray(bo, np.float32)
    gamma_np = np.asarray(gamma, dtype=np.float32)
    beta_np = np.asarray(beta, dtype=np.float32)

    wqk8, wv8, wob8, wot8, il8, bqk, r = _prep_weights(
        Wq, bq, Wk, bk, Wv, bv, Wo, bo
    )

    nc = _get_compiled()
    in_maps = []
    for bi in range(B):
        Xr = X[bi] + r
        X8 = Xr.astype(F8)
        # XT8[p, c, i, s] = X8[s, 256c + 2p + i]
        xt8 = np.ascontiguousarray(
            X8.reshape(S, NC, 128, 2).transpose(2, 1, 3, 0)
        )
        xbi = Xr.astype(BF)
        xbf = xbi.astype(np.float32)
        xb2 = (xbf ** 2).sum(axis=1)  # [S]
        xb2t = np.ascontiguousarray(xb2.reshape(NT, 128).T)  # [128, NT]
        xbs = xbf.sum(axis=1)
        xbst = np.ascontiguousarray(xbs.reshape(NT, 128).T)
        in_maps.append({
            "XT8": xt8, "XB": xbi, "XB2": xb2t, "XBS": xbst,
            "WQK8": wqk8, "WV8": wv8, "WOB8": wob8, "WOT8": wot8,
            "IL8": il8, "BQK": bqk,
        })
    res = run_bass_kernel_spmd(nc, in_maps, core_ids=list(range(B)))
    out = np.stack(
        [np.asarray(res.results[i]["OUT"], dtype=np.float32) for i in range(B)],
        axis=0,
    )
    if not (np.all(gamma_np == 1.0) and np.all(beta_np == 0.0)):
        out = out * gamma_np + beta_np
    return out.astype(np.float32)
